# revision 1
# baseline (speedup 1.0000x reference)
"""DANet head (dual attention: PAM + CAM) as an 8-core Trainium2 Bass kernel.

Sharding (one SPMD program on 8 cores, core id = cid):
  Phase 1: conv5a/conv5c (3x3, 2048->512) + BN + ReLU.  The 1024 combined
      output channels are split 128/core (cores 0-3: conv5a, 4-7: conv5c --
      same program, different per-core weight slices).  Each core also
      PE-transposes its slice; feat + featT slices are AllGathered.
  Phase 2a (PAM): sequence-parallel over query columns - core cid handles
      288 spatial positions per batch.  energy^T = k^T q computed column-
      sliced, exp'd without max-subtraction (|energy| < 30), U^T = expE^T-
      weighted v accumulated via PE, normalized by matmul-computed Z, then
      transposed back and fused with the residual.
  Phase 2b (CAM): channel rows 64*cid..64*cid+64 of the Gram-matrix
      attention for both batches; softmax via the rowmin identity
      softmax(rowmax-e) == exp(rowmin-e)/sum(exp(rowmin-e)).
  Phase 3: conv51/conv52 (3x3, 512->512) + BN + ReLU, 64+64 channels/core,
      run as interleaved M=64 matmul pairs over six concurrent psum tiles.
  Phase 4: per-core partial 1x1 convs for the three 19-channel outputs,
      AllReduce, bias add.

All matmuls run in float32r (tf32-like, full PE rate at free-dim >= 256).
"""

import numpy as np

import concourse.bass as bass
import concourse.tile as tile
from concourse import bacc, mybir
from concourse import bass_utils

F32 = mybir.dt.float32
F32R = mybir.dt.float32r
AX = mybir.AxisListType
ALU = mybir.AluOpType
ACT = mybir.ActivationFunctionType

N_CORES = 8
B = 2
CIN = 2048
CMID = 512
COUT = 19
HW = 48
N = HW * HW            # 2304
NPAD = 50 * 50         # 2500
EPS = 1e-5

FEAT = 128 * B * N             # 589824  elems, feat slice region
FEATT = B * N * 128            # 589824, transposed slice region
BLK = FEAT + FEATT             # per-rank ag1 payload
SA = B * CMID * 288            # 294912, sa_feat slice region
SC = B * 64 * N                # 294912, sc_feat slice region
AG2IN = SA + SC
AR = 3 * COUT * B * N          # 262656

OFFS = [(dy, dx) for dy in (-1, 0, 1) for dx in (-1, 0, 1)]
# (col0, width) tiles covering 2304 with width >= 256 (f32r full-rate)
NT5 = [(0, 512), (512, 512), (1024, 512), (1536, 512), (2048, 256)]
SUBS = [(0, 128), (128, 128), (256, 32)]   # 288 = 128+128+32


def _r(ap):
    return ap.bitcast(F32R)


def build_program(sim=False, reps=1):
    nc = bacc.Bacc("TRN2", target_bir_lowering=False, debug=False,
                   num_devices=1 if sim else N_CORES)

    # ---------------- kernel I/O ----------------
    x_pad = nc.dram_tensor("x_pad", [B, CIN, NPAD], F32, kind="ExternalInput")
    ident = nc.dram_tensor("ident", [128, 128], F32, kind="ExternalInput")
    w5s = nc.dram_tensor("w5s", [128, 16, 9, 128], F32, kind="ExternalInput")
    g5s = nc.dram_tensor("g5s", [128, 1], F32, kind="ExternalInput")
    b5s = nc.dram_tensor("b5s", [128, 1], F32, kind="ExternalInput")
    wqT = nc.dram_tensor("wqT", [4, 128, 64], F32, kind="ExternalInput")
    wkT = nc.dram_tensor("wkT", [4, 128, 64], F32, kind="ExternalInput")
    wvT = nc.dram_tensor("wvT", [4, 128, 512], F32, kind="ExternalInput")
    bq = nc.dram_tensor("bq", [64, 1], F32, kind="ExternalInput")
    bk = nc.dram_tensor("bk", [64, 1], F32, kind="ExternalInput")
    bv_row = nc.dram_tensor("bv_row", [1, 512], F32, kind="ExternalInput")
    g_pam = nc.dram_tensor("g_pam", [1, 1], F32, kind="ExternalInput")
    g_cam = nc.dram_tensor("g_cam", [1, 1], F32, kind="ExternalInput")
    w51s = nc.dram_tensor("w51s", [128, 4, 9, 64], F32, kind="ExternalInput")
    w52s = nc.dram_tensor("w52s", [128, 4, 9, 64], F32, kind="ExternalInput")
    g51s = nc.dram_tensor("g51s", [64, 1], F32, kind="ExternalInput")
    b51s = nc.dram_tensor("b51s", [64, 1], F32, kind="ExternalInput")
    g52s = nc.dram_tensor("g52s", [64, 1], F32, kind="ExternalInput")
    b52s = nc.dram_tensor("b52s", [64, 1], F32, kind="ExternalInput")
    w6Ts = nc.dram_tensor("w6Ts", [64, COUT], F32, kind="ExternalInput")
    w7Ts = nc.dram_tensor("w7Ts", [64, COUT], F32, kind="ExternalInput")
    w8Ts = nc.dram_tensor("w8Ts", [64, COUT], F32, kind="ExternalInput")
    b6 = nc.dram_tensor("b6", [COUT, 1], F32, kind="ExternalInput")
    b7 = nc.dram_tensor("b7", [COUT, 1], F32, kind="ExternalInput")
    b8 = nc.dram_tensor("b8", [COUT, 1], F32, kind="ExternalInput")
    out_sasc = nc.dram_tensor("out_sasc", [B, COUT, HW, HW], F32,
                              kind="ExternalOutput")
    out_sa = nc.dram_tensor("out_sa", [B, COUT, HW, HW], F32,
                            kind="ExternalOutput")
    out_sc = nc.dram_tensor("out_sc", [B, COUT, HW, HW], F32,
                            kind="ExternalOutput")

    with tile.TileContext(nc) as tc:
        with tc.tile_pool(name="dramp", bufs=1, space="DRAM") as dramp:

            with tc.tile_pool(name="consts", bufs=1) as consts:
                id_sb = consts.tile([128, 128], F32R)
                nc.sync.dma_start(out=id_sb, in_=_r(ident[:, :]))
                ones_sb = consts.tile([128, 1], F32)
                nc.vector.memset(ones_sb, 1.0)
                eps_sb = consts.tile([128, 1], F32)
                nc.vector.memset(eps_sb, EPS)
                gp_sb = consts.tile([128, 1], F32)
                nc.sync.dma_start(out=gp_sb, in_=g_pam[:, :].to_broadcast([128, 1]))
                gc_sb = consts.tile([128, 1], F32)
                nc.sync.dma_start(out=gc_sb, in_=g_cam[:, :].to_broadcast([128, 1]))

                for _rep in range(reps):
                    ag1_in = dramp.tile([BLK], F32)
                    ag1_out = dramp.tile([N_CORES * BLK], F32, addr_space="Shared")
                    ag2a_in = dramp.tile([SA], F32)
                    ag2a_out = dramp.tile([N_CORES * SA], F32, addr_space="Shared")
                    ag2b_in = dramp.tile([SC], F32)
                    ag2b_out = dramp.tile([N_CORES * SC], F32, addr_space="Shared")
                    ar_in = dramp.tile([AR], F32)
                    ar_out = dramp.tile([AR], F32, addr_space="Shared")

                    # per-core dynamic offsets (registers on the SP engine, which
                    # issues every dynamic DMA below); sim mode uses a fixed core id
                    # so TimelineSim (single-core, no collectives) can cost the
                    # program.
                    if sim:
                        pid = 3
                        n0 = pid * 288
                        camrow = (pid // 2) * 9216 + 4 * 9216 + 4608
                        camcol = (pid % 2) * 64
                        camx = (pid // 2) * 72 + 4 * 72 + 36
                        resrow = 4 * 256 + (pid // 2) * 256 + (pid % 2) * 64
                    else:
                        pid = nc.sync.partition_id()
                        n0 = nc.sync.snap(pid * 288, min_val=0, max_val=2016)
                        camrow = nc.sync.snap((pid // 2) * 9216 + 4 * 9216 + 4608,
                                              min_val=4 * 9216 + 4608,
                                              max_val=7 * 9216 + 4608)
                        camcol = nc.sync.snap((pid % 2) * 64, min_val=0, max_val=64)
                        camx = nc.sync.snap((pid // 2) * 72 + 4 * 72 + 36,
                                            min_val=4 * 72 + 36, max_val=7 * 72 + 36)
                        resrow = nc.sync.snap(4 * 256 + (pid // 2) * 256 + (pid % 2) * 64,
                                              min_val=4 * 256, max_val=7 * 256 + 64)

                    # flat views of the gathered buffers
                    v4608 = ag1_out[:].rearrange("(x c) -> x c", c=4608)   # feat rows
                    v128 = ag1_out[:].rearrange("(x c) -> x c", c=128)     # featT rows
                    featT_w = ag1_in[FEAT:BLK].rearrange("(b n c) -> b n c",
                                                         b=B, n=N, c=128)
                    sa_w = ag2a_in[:].rearrange("(b c n) -> b c n", b=B, c=CMID, n=288)
                    sc_w = ag2b_in[:].rearrange("(b c n) -> b c n", b=B, c=64, n=N)
                    vsa = ag2a_out[:].rearrange("(x n) -> x n", n=288)     # [8192, 288]
                    vsc = ag2b_out[:].rearrange("(x n) -> x n", n=N)       # [1024, 2304]
                    arv_r = ar_out[:].rearrange("(o c n) -> o c n", o=3, c=COUT, n=B * N)
                    # ======== Phase 1: conv5a/5c slice + BN + ReLU + transpose
                    with (
                        tc.tile_pool(name="p1w", bufs=1) as p1w,
                        tc.tile_pool(name="p1x", bufs=3) as p1x,
                        tc.tile_pool(name="p1f", bufs=1) as p1f,
                        tc.tile_pool(name="p1t", bufs=3) as p1t,
                        tc.tile_pool(name="p1ps", bufs=6, space="PSUM") as p1ps,
                        tc.tile_pool(name="p1tps", bufs=2, space="PSUM") as p1tps,
                    ):
                        w5_sb = p1w.tile([128, 16, 9, 128], F32R)
                        nc.sync.dma_start(out=w5_sb, in_=_r(w5s[:, :, :, :]))
                        g5_sb = p1f.tile([128, 1], F32, tag="g5")
                        nc.sync.dma_start(out=g5_sb, in_=g5s[:, :])
                        b5_sb = p1f.tile([128, 1], F32, tag="b5")
                        nc.sync.dma_start(out=b5_sb, in_=b5s[:, :])

                        feat_raw = p1f.tile([128, B, N], F32, tag="fraw")
                        stats = p1f.tile([128, 12, 6], F32, tag="stats")
                        feat_sb = p1f.tile([128, B, N], F32R, tag="feat")

                        for b in range(B):
                            pst = [p1ps.tile([128, 384], F32, tag="convps",
                                             name=f"c5ps_{b}_{t}")
                                   for t in range(6)]
                            for ch in range(16):
                                xs = p1x.tile([128, 50, 50], F32R, tag="xs",
                                              name=f"xs_{b}_{ch}")
                                nc.sync.dma_start(
                                    out=xs,
                                    in_=_r(x_pad[b, 128 * ch:128 * ch + 128, :]
                                           .rearrange("c (u v) -> c u v", v=50)))
                                for t in range(6):
                                    for oi, (dy, dx) in enumerate(OFFS):
                                        r0 = 8 * t + dy + 1
                                        nc.tensor.matmul(
                                            pst[t][:, :],
                                            lhsT=w5_sb[:, ch, oi, :],
                                            rhs=xs[:, r0:r0 + 8, dx + 1:dx + 49],
                                            start=(ch == 0 and oi == 0),
                                            stop=(ch == 15 and oi == 8))
                            for t in range(6):
                                nc.vector.tensor_copy(
                                    feat_raw[:, b, 384 * t:384 * t + 384], pst[t][:, :])
                                nc.vector.bn_stats(stats[:, 6 * b + t, :], pst[t][:, :])

                        mv = p1f.tile([128, 2], F32, tag="mv")
                        nc.vector.bn_aggr(mv, stats)
                        rstd = p1f.tile([128, 1], F32, tag="rstd")
                        nc.scalar.activation(rstd, mv[:, 1:2], ACT.Sqrt, bias=eps_sb)
                        nc.vector.reciprocal(rstd, rstd)
                        scale = p1f.tile([128, 1], F32, tag="scale")
                        nc.vector.tensor_tensor(scale, rstd, g5_sb, op=ALU.mult)
                        shift = p1f.tile([128, 1], F32, tag="shift")
                        nc.vector.tensor_tensor(shift, mv[:, 0:1], scale, op=ALU.mult)
                        nc.vector.tensor_tensor(shift, b5_sb, shift, op=ALU.subtract)
                        fr_flat = feat_raw.rearrange("p b n -> p (b n)")
                        nc.vector.tensor_scalar(fr_flat, fr_flat, scale, shift,
                                                op0=ALU.mult, op1=ALU.add)
                        nc.scalar.activation(feat_sb.rearrange("p b n -> p (b n)"),
                                             fr_flat, ACT.Relu)

                        nc.sync.dma_start(
                            out=_r(ag1_in[0:FEAT].rearrange("(p x) -> p x", p=128)),
                            in_=feat_sb.rearrange("p b n -> p (b n)"))
                        # ag1_in featT region as [x, p, c] row-blocks
                        ftv = ag1_in[FEAT:BLK].rearrange("(x p c) -> x p c",
                                                         p=128, c=128)
                        for b in range(B):
                            for g in range(3):
                                stg = p1t.tile([128, 6, 128], F32, tag="tstg",
                                               name=f"tstg_{b}_{g}")
                                for i in range(6):
                                    mt = 6 * g + i
                                    tp = p1tps.tile([128, 128], F32, tag="trps",
                                                    name=f"tr_{b}_{mt}")
                                    nc.tensor.transpose(
                                        _r(tp[:, :]),
                                        feat_sb[:, b, 128 * mt:128 * mt + 128],
                                        id_sb)
                                    nc.vector.tensor_copy(stg[:, i, :], tp)
                                nc.sync.dma_start(
                                    out=ftv[18 * b + 6 * g:18 * b + 6 * g + 6,
                                            :, :].transpose([1, 0, 2]),
                                    in_=stg)

                    if not sim:
                        nc.gpsimd.collective_compute(
                            "AllGather", ALU.bypass,
                            replica_groups=[list(range(N_CORES))],
                            ins=[ag1_in[:].opt()], outs=[ag1_out[:].opt()])

                    # ======== Phase 2a: PAM, my 288 query columns per batch
                    with (
                        tc.tile_pool(name="p2w", bufs=1) as p2w,
                        tc.tile_pool(name="p2f", bufs=1) as p2f,
                        tc.tile_pool(name="p2s", bufs=2) as p2s,
                        tc.tile_pool(name="psA", bufs=2, space="PSUM") as psA,
                        tc.tile_pool(name="psE", bufs=2, space="PSUM") as psE,
                        tc.tile_pool(name="psS", bufs=2, space="PSUM") as psS,
                        tc.tile_pool(name="psZ", bufs=1, space="PSUM") as psZ,
                        tc.tile_pool(name="psT", bufs=1, space="PSUM") as psT,
                    ):
                        wq_sb = p2w.tile([128, 4, 64], F32R)
                        wk_sb = p2w.tile([128, 4, 64], F32R)
                        wv_sb = p2w.tile([128, 4, 512], F32R)
                        for ch in range(4):
                            nc.sync.dma_start(out=wq_sb[:, ch, :], in_=_r(wqT[ch]))
                            nc.sync.dma_start(out=wk_sb[:, ch, :], in_=_r(wkT[ch]))
                            nc.sync.dma_start(out=wv_sb[:, ch, :], in_=_r(wvT[ch]))
                        bq_sb = p2w.tile([64, 1], F32)
                        nc.sync.dma_start(out=bq_sb, in_=bq[:, :])
                        bk_sb = p2w.tile([64, 1], F32)
                        nc.sync.dma_start(out=bk_sb, in_=bk[:, :])
                        bv_sb = p2w.tile([128, 512], F32)
                        nc.sync.dma_start(out=bv_sb,
                                          in_=bv_row[:, :].to_broadcast([128, 512]))
                        bv_bc = bv_sb[:, :]

                        va1 = ag1_out[:].rearrange("(j x c) -> j x c",
                                                    j=N_CORES, c=4608)
                        for b in range(B):
                            f1_all = p2f.tile([128, 4, N], F32R, tag="f1all", bufs=2,
                                              name=f"f1a_{b}")
                            nc.sync.dma_start(
                                out=_r(f1_all),
                                in_=_r(va1[0:4, 0:128, b * N:b * N + N]
                                       .transpose([1, 0, 2])))
                            f1 = [f1_all[:, j, :] for j in range(4)]
                            qr_all = p2f.tile([128, 4, 288], F32R, tag="qrall",
                                              name=f"qra_{b}")
                            nc.sync.dma_start(
                                out=_r(qr_all),
                                in_=_r(va1[0:4, 0:128, bass.ds(b * N + n0, 288)]
                                       .transpose([1, 0, 2])))
                            qr = [qr_all[:, j, :] for j in range(4)]

                            k_sb = p2f.tile([64, N], F32R, tag="k", name=f"k_{b}")
                            for (c0, cn) in NT5:
                                kp = psS.tile([64, cn], F32, tag="sps",
                                              name=f"kp_{b}_{c0}")
                                for ch in range(4):
                                    nc.tensor.matmul(kp, lhsT=wk_sb[:, ch, :],
                                                     rhs=f1[ch][:, c0:c0 + cn],
                                                     start=(ch == 0), stop=(ch == 3))
                                nc.vector.tensor_scalar(
                                    _r(k_sb[:, c0:c0 + cn]), kp, bk_sb, None,
                                    op0=ALU.add)
                            qp = psS.tile([64, 288], F32, tag="sps", name=f"qp_{b}")
                            for ch in range(4):
                                nc.tensor.matmul(qp, lhsT=wq_sb[:, ch, :],
                                                 rhs=qr[ch],
                                                 start=(ch == 0), stop=(ch == 3))
                            q_sb = p2f.tile([64, 288], F32R, tag="q", name=f"q_{b}")
                            nc.vector.tensor_scalar(_r(q_sb[:, :]), qp, bq_sb, None,
                                                    op0=ALU.add)

                            vts = []
                            for m in range(18):
                                vp = psA.tile([128, 512], F32, tag="bigps",
                                              name=f"vp_{b}_{m}")
                                for ch in range(4):
                                    nc.tensor.matmul(vp,
                                                     lhsT=f1[ch][:, 128 * m:128 * m + 128],
                                                     rhs=wv_sb[:, ch, :],
                                                     start=(ch == 0), stop=(ch == 3))
                                vt = p2f.tile([128, 512], F32R, tag=f"vt_{m}",
                                              name=f"vt_{b}_{m}")
                                nc.vector.tensor_tensor(_r(vt[:, :]), vp, bv_bc,
                                                        op=ALU.add)
                                vts.append(vt)

                            exs = []
                            for m in range(18):
                                ep = psE.tile([128, 288], F32, tag="eps",
                                              name=f"ep_{b}_{m}")
                                nc.tensor.matmul(ep,
                                                 lhsT=k_sb[:, 128 * m:128 * m + 128],
                                                 rhs=q_sb,
                                                 start=True, stop=True)
                                ex = p2f.tile([128, 288], F32R, tag=f"ex_{m}",
                                              name=f"ex_{b}_{m}")
                                nc.scalar.activation(_r(ex[:, :]), ep, ACT.Exp)
                                exs.append(ex)

                            sa_sb = [p2s.tile([128, 288], F32, tag=f"sa_{cc}",
                                              name=f"sa_{b}_{cc}")
                                     for cc in range(4)]
                            for (s0, sn) in SUBS:
                                up = psA.tile([128, 512], F32, tag="bigps",
                                              name=f"up_{b}_{s0}")
                                zp = psZ.tile([128, 1], F32, tag="zps",
                                              name=f"zp_{b}_{s0}")
                                for m in range(18):
                                    nc.tensor.matmul(up[0:sn, :],
                                                     lhsT=exs[m][:, s0:s0 + sn],
                                                     rhs=vts[m],
                                                     start=(m == 0), stop=(m == 17))
                                    nc.tensor.matmul(zp[0:sn, :],
                                                     lhsT=exs[m][:, s0:s0 + sn]
                                                     .bitcast(F32),
                                                     rhs=ones_sb,
                                                     start=(m == 0), stop=(m == 17))
                                rz = p2s.tile([128, 1], F32, tag="rz",
                                              name=f"rz_{b}_{s0}")
                                nc.vector.reciprocal(rz[0:sn, :], zp[0:sn, :])
                                nc.vector.tensor_tensor(rz[0:sn, :], rz[0:sn, :],
                                                        gp_sb[0:sn, :], op=ALU.mult)
                                u_sb = p2s.tile([128, 512], F32R, tag="usb",
                                                name=f"u_{b}_{s0}")
                                nc.vector.tensor_scalar(
                                    _r(u_sb[0:sn, :]), up[0:sn, :], rz[0:sn, :],
                                    None, op0=ALU.mult)
                                for cc in range(4):
                                    tp = psT.tile([128, 128], F32, tag="tps",
                                                  name=f"utp_{b}_{s0}_{cc}")
                                    nc.tensor.transpose(
                                        _r(tp[:, 0:sn]),
                                        u_sb[0:sn, 128 * cc:128 * cc + 128],
                                        id_sb[0:sn, 0:sn])
                                    nc.vector.tensor_tensor(
                                        sa_sb[cc][:, s0:s0 + sn], tp[:, 0:sn],
                                        qr[cc][:, s0:s0 + sn], op=ALU.add)
                            for cc in range(4):
                                nc.sync.dma_start(
                                    out=sa_w[b, 128 * cc:128 * cc + 128, :],
                                    in_=sa_sb[cc])

                    if not sim:
                        # sa-half gather overlaps the whole CAM phase
                        nc.gpsimd.collective_compute(
                            "AllGather", ALU.bypass,
                            replica_groups=[list(range(N_CORES))],
                            ins=[ag2a_in[:].opt()], outs=[ag2a_out[:].opt()])

                    # ======== Phase 2b: CAM, my 64 channel rows per batch
                    with (
                        tc.tile_pool(name="p2c", bufs=1) as p2c,
                        tc.tile_pool(name="p2cs", bufs=2) as p2cs,
                        tc.tile_pool(name="psB", bufs=2, space="PSUM") as psB,
                        tc.tile_pool(name="psT2", bufs=2, space="PSUM") as psT2,
                    ):
                        # [x, p, c] view of ag1_out for whole-row-block gathers
                        vm = ag1_out[:].rearrange("(x p c) -> x p c", p=128, c=128)
                        for b in range(B):
                            # featT rows of blocks 4..7, batch b, as one 4D gather
                            xft_all = p2c.tile([128, 18, 4, 128], F32R,
                                               tag="xftall", name=f"xfta_{b}")
                            for j in range(4):
                                x0 = ((4 + j) * 9216 + 4608 + b * N) // 128
                                nc.sync.dma_start(
                                    out=_r(xft_all[:, :, j, :]),
                                    in_=_r(vm[x0:x0 + 18, :, :]
                                           .transpose([1, 0, 2])))
                            xfts = [xft_all[:, m, :, :] for m in range(18)]
                            lc_all = p2c.tile([128, 18, 64], F32R,
                                              tag="lcall", name=f"lca_{b}")
                            srcl = vm[bass.ds(camx + 18 * b, 18), :,
                                      bass.ds(camcol, 64)].transpose([1, 0, 2])
                            nc.sync.dma_start(out=_r(lc_all), in_=_r(srcl))
                            lcs = [lc_all[:, m, :] for m in range(18)]
                            f2n_all = p2c.tile([128, 4, N], F32R, tag="f2nall",
                                               name=f"f2na_{b}")
                            nc.sync.dma_start(
                                out=_r(f2n_all),
                                in_=_r(va1[4:8, 0:128, b * N:b * N + N]
                                       .transpose([1, 0, 2])))
                            f2n = [f2n_all[:, j, :] for j in range(4)]
                            res_sb = p2c.tile([64, N], F32, tag="res",
                                              name=f"res_{b}")
                            nc.sync.dma_start(
                                out=res_sb,
                                in_=v4608[bass.ds(resrow, 64), b * N:b * N + N])

                            e2p = psB.tile([64, 512], F32, tag="e2ps",
                                           name=f"e2p_{b}")
                            for m in range(18):
                                nc.tensor.matmul(e2p, lhsT=lcs[m], rhs=xfts[m],
                                                 start=(m == 0), stop=(m == 17))
                            rmin = p2cs.tile([64, 1], F32, tag="rmin",
                                             name=f"rmin_{b}")
                            nc.vector.tensor_reduce(rmin, e2p, axis=AX.X, op=ALU.min)
                            attn = p2cs.tile([64, 512], F32, tag="attn",
                                             name=f"attn_{b}")
                            rsum = p2cs.tile([64, 1], F32, tag="rsum",
                                             name=f"rsum_{b}")
                            nc.scalar.activation(attn, e2p, ACT.Exp,
                                                 bias=rmin, scale=-1.0,
                                                 accum_out=rsum)
                            nc.vector.reciprocal(rsum, rsum)
                            nc.vector.tensor_tensor(rsum, rsum, gc_sb[0:64, :],
                                                    op=ALU.mult)
                            attn2 = p2cs.tile([64, 512], F32R, tag="attn2",
                                              name=f"attn2_{b}")
                            nc.vector.tensor_scalar(_r(attn2[:, :]), attn, rsum,
                                                    None, op0=ALU.mult)
                            atT = []
                            for j in range(4):
                                tp = psT2.tile([128, 128], F32, tag="t2ps",
                                               name=f"atp_{b}_{j}")
                                nc.tensor.transpose(_r(tp[:, 0:64]),
                                                    attn2[:, 128 * j:128 * j + 128],
                                                    id_sb[0:64, 0:64])
                                t = p2c.tile([128, 64], F32R, tag=f"at_{j}",
                                             name=f"at_{b}_{j}")
                                nc.vector.tensor_copy(_r(t[:, :]), tp[:, 0:64])
                                atT.append(t)
                            sc_sb = p2c.tile([64, N], F32, tag="scout",
                                             name=f"sc_{b}")
                            for (c0, cn) in NT5:
                                op = psB.tile([64, cn], F32, tag="e2ps",
                                              name=f"op_{b}_{c0}")
                                for j in range(4):
                                    nc.tensor.matmul(op, lhsT=atT[j],
                                                     rhs=f2n[j][:, c0:c0 + cn],
                                                     start=(j == 0), stop=(j == 3))
                                nc.vector.tensor_tensor(sc_sb[:, c0:c0 + cn], op,
                                                        res_sb[:, c0:c0 + cn],
                                                        op=ALU.add)
                            nc.sync.dma_start(out=sc_w[b, :, :], in_=sc_sb)

                    if not sim:
                        nc.gpsimd.collective_compute(
                            "AllGather", ALU.bypass,
                            replica_groups=[list(range(N_CORES))],
                            ins=[ag2b_in[:].opt()], outs=[ag2b_out[:].opt()])

                    # ======== Phase 3: conv51+conv52 (64+64 ch, M=64 pairs) + BN
                    # image-row runs per half-window: (block, img_row0, n_rows)
                    SA_RUNS = {
                        0: [(0, 0, 6), (1, 6, 6), (2, 12, 6), (3, 18, 6), (4, 24, 1)],
                        1: [(3, 23, 1), (4, 24, 6), (5, 30, 6), (6, 36, 6), (7, 42, 6)],
                    }
                    SC_COLS = {0: (0, 0, 25), 1: (23, 1104, 25)}  # (imgrow0, col0, nrows)
                    with (
                        tc.tile_pool(name="p3w", bufs=1) as p3w,
                        tc.tile_pool(name="p3x", bufs=1) as p3x,
                        tc.tile_pool(name="p3f", bufs=1) as p3f,
                    ):
                        w51_sb = p3w.tile([128, 4, 9, 64], F32R)
                        nc.sync.dma_start(out=w51_sb, in_=_r(w51s[:, :, :, :]))
                        w52_sb = p3w.tile([128, 4, 9, 64], F32R)
                        nc.sync.dma_start(out=w52_sb, in_=_r(w52s[:, :, :, :]))
                        g51_sb = p3f.tile([64, 1], F32, tag="g51")
                        nc.sync.dma_start(out=g51_sb, in_=g51s[:, :])
                        b51_sb = p3f.tile([64, 1], F32, tag="b51")
                        nc.sync.dma_start(out=b51_sb, in_=b51s[:, :])
                        g52_sb = p3f.tile([64, 1], F32, tag="g52")
                        nc.sync.dma_start(out=g52_sb, in_=g52s[:, :])
                        b52_sb = p3f.tile([64, 1], F32, tag="b52")
                        nc.sync.dma_start(out=b52_sb, in_=b52s[:, :])

                        # stage tiles keyed by window-half (pad rows differ) and
                        # double-buffered over the cin-chunk loop; pads stay zero
                        # after a single memset because loads only touch the
                        # interior.
                        stage_sa = [[p3x.tile([128, 26, 50], F32,
                                              tag=f"ssa{th}{k}", name=f"ssa{th}{k}")
                                     for k in range(2)] for th in range(2)]
                        stage_sc = [[p3x.tile([128, 26, 50], F32,
                                              tag=f"ssc{th}{k}", name=f"ssc{th}{k}")
                                     for k in range(2)] for th in range(2)]
                        for th in range(2):
                            for k in range(2):
                                nc.gpsimd.memset(stage_sa[th][k], 0.0)
                                nc.gpsimd.memset(stage_sc[th][k], 0.0)

                        c51_raw = p3f.tile([64, B, N], F32, tag="c51raw")
                        c52_raw = p3f.tile([64, B, N], F32, tag="c52raw")
                        st51 = p3f.tile([64, 12, 6], F32, tag="st51")
                        st52 = p3f.tile([64, 12, 6], F32, tag="st52")

                        with tc.tile_pool(name="p3ps", bufs=7, space="PSUM") as p3ps:
                            for b in range(B):
                                for th in range(2):
                                    pst = [p3ps.tile([64, 384], F32, tag="c3ps",
                                                     name=f"c3ps_{b}_{th}_{t}")
                                           for t in range(6)]
                                    # conv51 first: depends only on the
                                    # sa-gather, which overlapped CAM; the
                                    # sc-gather may still be in flight.
                                    for ch in range(4):
                                        ssa = stage_sa[th][ch % 2]
                                        for (bl, ir0, nr) in SA_RUNS[th]:
                                            lr0 = ir0 - 6 * bl
                                            s0 = ir0 - (24 * th - 1)
                                            row = 1024 * bl + 512 * b + 128 * ch
                                            nc.sync.dma_start(
                                                out=_r(ssa[:, s0:s0 + nr, 1:49]),
                                                in_=_r(vsa[row:row + 128,
                                                           48 * lr0:48 * (lr0 + nr)]
                                                       .rearrange("p (u v) -> p u v",
                                                                  v=48)))
                                        for t3 in range(3):
                                            for oi, (dy, dx) in enumerate(OFFS):
                                                r0 = 8 * t3 + dy + 1
                                                nc.tensor.matmul(
                                                    pst[t3][:, :],
                                                    lhsT=w51_sb[:, ch, oi, :],
                                                    rhs=_r(ssa[:, r0:r0 + 8,
                                                               dx + 1:dx + 49]),
                                                    start=(ch == 0 and oi == 0),
                                                    stop=(ch == 3 and oi == 8))
                                    for ch in range(4):
                                        ssc = stage_sc[th][ch % 2]
                                        (ir0, col0, nrows) = SC_COLS[th]
                                        s0 = ir0 - (24 * th - 1)
                                        for half in range(2):
                                            bl = 2 * ch + half
                                            row = 128 * bl + 64 * b
                                            nc.sync.dma_start(
                                                out=_r(ssc[64 * half:64 * half + 64,
                                                           s0:s0 + nrows, 1:49]),
                                                in_=_r(vsc[row:row + 64,
                                                           col0:col0 + 48 * nrows]
                                                       .rearrange("p (u v) -> p u v",
                                                                  v=48)))
                                        for t3 in range(3):
                                            for oi, (dy, dx) in enumerate(OFFS):
                                                r0 = 8 * t3 + dy + 1
                                                nc.tensor.matmul(
                                                    pst[3 + t3][:, :],
                                                    lhsT=w52_sb[:, ch, oi, :],
                                                    rhs=_r(ssc[:, r0:r0 + 8,
                                                               dx + 1:dx + 49]),
                                                    start=(ch == 0 and oi == 0),
                                                    stop=(ch == 3 and oi == 8))
                                    for t3 in range(3):
                                        col0 = 384 * (3 * th + t3)
                                        g = 6 * b + 3 * th + t3
                                        nc.vector.tensor_copy(
                                            c51_raw[:, b, col0:col0 + 384],
                                            pst[t3][:, :])
                                        nc.vector.bn_stats(st51[:, g, :],
                                                           pst[t3][:, :])
                                        nc.vector.tensor_copy(
                                            c52_raw[:, b, col0:col0 + 384],
                                            pst[3 + t3][:, :])
                                        nc.vector.bn_stats(st52[:, g, :],
                                                           pst[3 + t3][:, :])

                        mv51 = p3f.tile([64, 2], F32, tag="mv51")
                        nc.vector.bn_aggr(mv51, st51)
                        rstd51 = p3f.tile([64, 1], F32, tag="rstd51")
                        nc.scalar.activation(rstd51, mv51[:, 1:2], ACT.Sqrt,
                                             bias=eps_sb[0:64, :])
                        nc.vector.reciprocal(rstd51, rstd51)
                        scale51 = p3f.tile([64, 1], F32, tag="scale51")
                        nc.vector.tensor_tensor(scale51, rstd51, g51_sb, op=ALU.mult)
                        shift51 = p3f.tile([64, 1], F32, tag="shift51")
                        nc.vector.tensor_tensor(shift51, mv51[:, 0:1], scale51,
                                                op=ALU.mult)
                        nc.vector.tensor_tensor(shift51, b51_sb, shift51,
                                                op=ALU.subtract)
                        cr51 = c51_raw.rearrange("p b n -> p (b n)")
                        nc.vector.tensor_scalar(cr51, cr51, scale51, shift51,
                                                op0=ALU.mult, op1=ALU.add)
                        sa_conv = p3f.tile([64, B * N], F32R, tag="sa_conv")
                        nc.scalar.activation(sa_conv, cr51, ACT.Relu)
                        mv52 = p3f.tile([64, 2], F32, tag="mv52")
                        nc.vector.bn_aggr(mv52, st52)
                        rstd52 = p3f.tile([64, 1], F32, tag="rstd52")
                        nc.scalar.activation(rstd52, mv52[:, 1:2], ACT.Sqrt,
                                             bias=eps_sb[0:64, :])
                        nc.vector.reciprocal(rstd52, rstd52)
                        scale52 = p3f.tile([64, 1], F32, tag="scale52")
                        nc.vector.tensor_tensor(scale52, rstd52, g52_sb, op=ALU.mult)
                        shift52 = p3f.tile([64, 1], F32, tag="shift52")
                        nc.vector.tensor_tensor(shift52, mv52[:, 0:1], scale52,
                                                op=ALU.mult)
                        nc.vector.tensor_tensor(shift52, b52_sb, shift52,
                                                op=ALU.subtract)
                        cr52 = c52_raw.rearrange("p b n -> p (b n)")
                        nc.vector.tensor_scalar(cr52, cr52, scale52, shift52,
                                                op0=ALU.mult, op1=ALU.add)
                        sc_conv = p3f.tile([64, B * N], F32R, tag="sc_conv")
                        nc.scalar.activation(sc_conv, cr52, ACT.Relu)

                        # ---- Phase 4: partial 1x1 convs + AllReduce + bias
                        with tc.tile_pool(name="p4ps", bufs=6, space="PSUM") as p4ps:
                            w6_sb = p3w.tile([64, COUT], F32R, tag="w6")
                            nc.sync.dma_start(out=w6_sb, in_=_r(w6Ts[:, :]))
                            w7_sb = p3w.tile([64, COUT], F32R, tag="w7")
                            nc.sync.dma_start(out=w7_sb, in_=_r(w7Ts[:, :]))
                            w8_sb = p3w.tile([64, COUT], F32R, tag="w8")
                            nc.sync.dma_start(out=w8_sb, in_=_r(w8Ts[:, :]))
                            arv_w = ar_in[:].rearrange("(o c n) -> o c n",
                                                       o=3, c=COUT, n=B * N)
                            plans = [[(w6_sb, sa_conv)], [(w7_sb, sc_conv)],
                                     [(w8_sb, sa_conv), (w8_sb, sc_conv)]]
                            for o, plan in enumerate(plans):
                                pstage = p3f.tile([COUT, B * N], F32, tag="pstage",
                                                  bufs=1, name=f"pstage_{o}")
                                for nt in range(9):
                                    c0 = 512 * nt
                                    pp = p4ps.tile([COUT, 512], F32, tag="pps",
                                                   name=f"pp_{nt}_{o}")
                                    for pi, (wt, conv) in enumerate(plan):
                                        nc.tensor.matmul(
                                            pp, lhsT=wt, rhs=conv[:, c0:c0 + 512],
                                            start=(pi == 0),
                                            stop=(pi == len(plan) - 1))
                                    nc.vector.tensor_copy(pstage[:, c0:c0 + 512], pp)
                                nc.sync.dma_start(out=arv_w[o], in_=pstage)

                            if not sim:
                                nc.gpsimd.collective_compute(
                                    "AllReduce", ALU.add,
                                    replica_groups=[list(range(N_CORES))],
                                    ins=[ar_in[:].opt()], outs=[ar_out[:].opt()])

                            # arv slot 0 = w6-partials (sa head), 1 = w7 (sc head),
                            # 2 = w8 (sasc head)
                            finals = [(2, b8, out_sasc), (0, b6, out_sa),
                                      (1, b7, out_sc)]
                            for (o, bias_t, out_t) in finals:
                                bias_sb = p3f.tile([COUT, 1], F32, tag="biasf",
                                                   bufs=3, name=f"bias_{o}")
                                nc.sync.dma_start(out=bias_sb, in_=bias_t[:, :])
                                fo = p3f.tile([COUT, B, N], F32, tag="fo",
                                              bufs=1, name=f"fo_{o}")
                                nc.sync.dma_start(
                                    out=fo,
                                    in_=arv_r[o].rearrange("c (b n) -> c b n", b=B))
                                nc.vector.tensor_scalar(fo, fo, bias_sb, None,
                                                        op0=ALU.add)
                                for b in range(B):
                                    nc.sync.dma_start(
                                        out=out_t[b].rearrange("c u v -> c (u v)"),
                                        in_=fo[:, b, :])
    nc.compile()
    return nc


# ---------------------------------------------------------------------------
# host side
# ---------------------------------------------------------------------------

_PROGRAM = None


def _prep_in_maps(inputs):
    f32 = np.float32
    x = np.asarray(inputs["x"], f32)
    xp = np.zeros((B, CIN, 50, 50), f32)
    xp[:, :, 1:49, 1:49] = x
    xp = xp.reshape(B, CIN, NPAD)

    def conv_w(w):                       # [co, ci, 3, 3] -> [ci%128, ci//128, 9, co]
        co, ci = w.shape[0], w.shape[1]
        a = w.transpose(1, 2, 3, 0).reshape(ci // 128, 128, 9, co)
        return np.ascontiguousarray(a.transpose(1, 0, 2, 3))

    wq = np.asarray(inputs["wq"], f32)[:, :, 0, 0]
    wk = np.asarray(inputs["wk"], f32)[:, :, 0, 0]
    wv = np.asarray(inputs["wv"], f32)[:, :, 0, 0]
    common = {
        "x_pad": xp,
        "ident": np.eye(128, dtype=f32),
        "wqT": np.ascontiguousarray(wq.T.reshape(4, 128, 64)),
        "wkT": np.ascontiguousarray(wk.T.reshape(4, 128, 64)),
        "wvT": np.ascontiguousarray(wv.T.reshape(4, 128, 512)),
        "bq": np.asarray(inputs["bq"], f32).reshape(64, 1),
        "bk": np.asarray(inputs["bk"], f32).reshape(64, 1),
        "bv_row": np.asarray(inputs["bv"], f32).reshape(1, 512),
        "g_pam": np.asarray(inputs["gamma_pam"], f32).reshape(1, 1),
        "g_cam": np.asarray(inputs["gamma_cam"], f32).reshape(1, 1),
        "b6": np.asarray(inputs["b6"], f32).reshape(COUT, 1),
        "b7": np.asarray(inputs["b7"], f32).reshape(COUT, 1),
        "b8": np.asarray(inputs["b8"], f32).reshape(COUT, 1),
    }
    w5a = np.asarray(inputs["w5a"], f32)
    w5c = np.asarray(inputs["w5c"], f32)
    w51 = np.asarray(inputs["w51"], f32)
    w52 = np.asarray(inputs["w52"], f32)
    w6 = np.asarray(inputs["w6"], f32)[:, :, 0, 0]
    w7 = np.asarray(inputs["w7"], f32)[:, :, 0, 0]
    w8 = np.asarray(inputs["w8"], f32)[:, :, 0, 0]
    g5a = np.asarray(inputs["g5a"], f32)
    b5a = np.asarray(inputs["b5a"], f32)
    g5c = np.asarray(inputs["g5c"], f32)
    b5c = np.asarray(inputs["b5c"], f32)
    g51 = np.asarray(inputs["g51"], f32)
    b51 = np.asarray(inputs["b51"], f32)
    g52 = np.asarray(inputs["g52"], f32)
    b52 = np.asarray(inputs["b52"], f32)

    in_maps = []
    for i in range(N_CORES):
        if i < 4:
            W1, gg, bb = w5a[128 * i:128 * i + 128], \
                g5a[128 * i:128 * i + 128], b5a[128 * i:128 * i + 128]
        else:
            j = i - 4
            W1, gg, bb = w5c[128 * j:128 * j + 128], \
                g5c[128 * j:128 * j + 128], b5c[128 * j:128 * j + 128]
        s = slice(64 * i, 64 * i + 64)
        m = dict(common)
        m.update({
            "w5s": conv_w(W1),
            "g5s": gg.reshape(128, 1),
            "b5s": bb.reshape(128, 1),
            "w51s": conv_w(w51[s]),
            "w52s": conv_w(w52[s]),
            "g51s": g51[s].reshape(64, 1),
            "b51s": b51[s].reshape(64, 1),
            "g52s": g52[s].reshape(64, 1),
            "b52s": b52[s].reshape(64, 1),
            "w6Ts": np.ascontiguousarray(w6[:, s].T),
            "w7Ts": np.ascontiguousarray(w7[:, s].T),
            "w8Ts": np.ascontiguousarray(w8[:, s].T),
        })
        in_maps.append(m)
    return in_maps


def get_program():
    global _PROGRAM
    if _PROGRAM is None:
        _PROGRAM = build_program()
    return _PROGRAM


def kernel(**inputs):
    nc = get_program()
    in_maps = _prep_in_maps(inputs)
    res = bass_utils.run_bass_kernel_spmd(nc, in_maps,
                                          core_ids=list(range(N_CORES)))
    r = res.results[0]
    shape = (B, COUT, HW, HW)
    return (r["out_sasc"].reshape(shape).astype(np.float32),
            r["out_sa"].reshape(shape).astype(np.float32),
            r["out_sc"].reshape(shape).astype(np.float32))


if __name__ == "__main__":
    import reference as R
    inp = {k: np.asarray(v) for k, v in R.setup_inputs().items()}
    got = kernel(**inp)
    print("kernel ran; shapes:", [g.shape for g in got])

